# revision 17
# baseline (speedup 1.0000x reference)
"""Distributed volume-argmin (correlation volume max/argmax) on 8 NeuronCores.

Problem: feat_l, feat_r [1, 128, 96, 128] fp32.
  corr[b,h,w,ij] = sum_c feat_l[b,c,h,w] * feat_r[b,c,i,j]
  flow_cost = max over ij, flat = argmax over ij,
  flow = (xoff - (flat % W_R)*scale_x, yoff - (flat // W_R)*scale_y)

Sharding: the 96 left-image rows are split across 8 cores (12 rows = 1536
left pixels each); feat_r is replicated. Each core computes its
[1536 x 12288] correlation block (K=C=128 contraction on partitions) and a
full per-pixel max/argmax, so no cross-core reduction is needed — outputs
concatenate.

Matmul precision: inputs are split host-side into bf16 hi/lo pairs and each
512-chunk accumulates hi*hi + hi*lo + lo*hi in fp32 PSUM (fp32-grade
accuracy at 3x-bf16-pass cost).

Argmax strategy per 128-pixel tile (partition = left pixel, free = 12288
right pixels): a ternary-then-binary fold tree on the Vector engine.
The 12288 row is split in thirds T1|T2|U; U (groups 4,5) is evicted
PSUM->SBUF by ScalarE (x2), then
  cma[j] = max(T2[j] (PSUM), x2[j])         j in [0,4096)
  cm1[j] = max(T1[j] (PSUM), cma[j])
  cm2[j] = max(cm1[j], cm1[j+2048])
  cm3[j] = max(cm2[j], cm2[j+1024])
so the DVE folds double as the PSUM drain. MAX8 + FIND_INDEX8 on the
1024-wide cm3 give the exact max M and the fold residue i*. The fold
branch bits are recovered exactly on the Scalar engine with its free
sum-accumulator: count = sum_j sign(M - y[j]) over y in {x2, cma,
cm1_hi, cm2_hi} equals the array size minus 1 exactly when M lives in
that array (sign(0)=0 on HW, verified); flat = base + 2048*b + 1024*c
+ i* with base = 8192 if M in U else 4096 if M in T2|U else 0. All
values are exact fp32 throughout, and ties resolve like jnp.argmax
(first occurrence) because corr maxima are unique for continuous data.
"""

import sys

for p in ("/opt/trn_rl_repo",):
    if p not in sys.path:
        sys.path.insert(0, p)

import numpy as np
import ml_dtypes

import concourse.bass as bass
import concourse.tile as tile
from concourse import bacc, mybir
from concourse.bass_utils import run_bass_kernel_spmd

# Problem geometry (hardcoded per the task contract).
B, C, H, W = 1, 128, 96, 128
H_R, W_R = 96, 128
NPIX = H_R * W_R              # 12288 right pixels
NCORES = 8
ROWS_PER_CORE = H // NCORES   # 12 left rows per core
NT = ROWS_PER_CORE            # one tile per left row (128 pixels)
CHUNK = 512
GRP = 1024                    # psum group (2 banks)
NGRP = NPIX // GRP            # 6
WT = NPIX // 3               # third width: 4096 (ternary first level)
W2F = WT // 2                 # 2048 (fold2 output)
W3F = WT // 4                 # 1024 (fold3 output)

_F32 = mybir.dt.float32
_BF16 = mybir.dt.bfloat16
_U32 = mybir.dt.uint32
_FP8 = mybir.dt.float8e4

_compiled = {}


def _emit_group_mms(nc, ps_ap, lh, ll, rh_t, rl_t, local_base):
    """3-pass bf16 matmuls for the chunks of one psum group.

    Pass-major order (hi*hi, hi*lo, lo*hi across chunks) so consecutive
    matmuls write different PSUM banks — same-bank accumulation
    back-to-back serializes the PE's fill/drain pipeline (379 vs 216 ns).
    rh_t/rl_t are per-third rhs tiles; local_base is the column offset of
    this group within its third."""
    for pass_lhs, pass_rhs, st, sp in (
        (lh, rh_t, True, False),
        (lh, rl_t, False, False),
        (ll, rh_t, False, True),
    ):
        for s in range(GRP // CHUNK):
            col0 = local_base + s * CHUNK
            out = ps_ap[:, s * CHUNK:(s + 1) * CHUNK]
            nc.tensor.matmul(out, pass_lhs, pass_rhs[:, col0:col0 + CHUNK],
                             start=st, stop=sp)


def _build_nc():
    nc = bacc.Bacc("TRN2", target_bir_lowering=False, debug=False,
                   num_devices=NCORES)

    lhs_hi_d = nc.declare_dram_parameter("lhs_hi", [C, NT * 128], _BF16,
                                         isOutput=False)
    lhs_lo_d = nc.declare_dram_parameter("lhs_lo", [C, NT * 128], _BF16,
                                         isOutput=False)
    rhs_hi_d = nc.declare_dram_parameter("rhs_hi", [C, NPIX], _BF16,
                                         isOutput=False)
    rhs_lo_d = nc.declare_dram_parameter("rhs_lo", [C, NPIX], _BF16,
                                         isOutput=False)
    cost_d = nc.declare_dram_parameter("cost", [128, NT], _F32, isOutput=True)
    idx_d = nc.declare_dram_parameter("idx", [128, NT], _U32, isOutput=True)
    cntu_d = nc.declare_dram_parameter("cntu", [128, NT], _F32, isOutput=True)
    cnta_d = nc.declare_dram_parameter("cnta", [128, NT], _F32, isOutput=True)
    cntb_d = nc.declare_dram_parameter("cntb", [128, NT], _F32, isOutput=True)
    cntc_d = nc.declare_dram_parameter("cntc", [128, NT], _F32, isOutput=True)

    with tile.TileContext(nc) as tc:
        with (
            tc.tile_pool(name="rhs", bufs=1) as rhs_pool,
            tc.tile_pool(name="lhs", bufs=1) as lhs_pool,
            tc.tile_pool(name="x2p", bufs=2) as x2_pool,
            tc.tile_pool(name="cmap", bufs=2) as cma_pool,
            tc.tile_pool(name="cm1p", bufs=2) as cm1_pool,
            tc.tile_pool(name="cm2p", bufs=1) as cm2_pool,
            tc.tile_pool(name="cm3p", bufs=2) as cm3_pool,
            tc.tile_pool(name="dumpp", bufs=1) as dump_pool,
            tc.tile_pool(name="m8p", bufs=3) as m8_pool,
            tc.tile_pool(name="ps", bufs=4, space="PSUM") as ps_pool,
            tc.tile_pool(name="outcols", bufs=1) as out_pool,
        ):
            lhs_hi = lhs_pool.tile([C, NT * 128], _BF16, tag="lh")
            lhs_lo = lhs_pool.tile([C, NT * 128], _BF16, tag="ll")
            # per-third rhs tiles: Tile tracks DMA dependencies per tile, so
            # the first matmuls wait only for their own third's load instead
            # of the whole 6.3 MB rhs. hi via sync (HWDGE), lo via gpsimd
            # (SWDGE). Load order: U third (needed first), then T2, then T1.
            rh0 = rhs_pool.tile([C, WT], _BF16, tag="rh0")
            rh1 = rhs_pool.tile([C, WT], _BF16, tag="rh1")
            rh2 = rhs_pool.tile([C, WT], _BF16, tag="rh2")
            rl0 = rhs_pool.tile([C, WT], _BF16, tag="rl0")
            rl1 = rhs_pool.tile([C, WT], _BF16, tag="rl1")
            rl2 = rhs_pool.tile([C, WT], _BF16, tag="rl2")
            rhs_hi_T = [rh0, rh1, rh2]
            rhs_lo_T = [rl0, rl1, rl2]
            nc.sync.dma_start(rhs_hi_T[2][:], rhs_hi_d[:, 2 * WT:3 * WT])
            nc.sync.dma_start(lhs_hi[:], lhs_hi_d[:])
            nc.sync.dma_start(rhs_lo_T[2][:], rhs_lo_d[:, 2 * WT:3 * WT])
            nc.sync.dma_start(lhs_lo[:], lhs_lo_d[:])
            for i in (1, 0):
                nc.sync.dma_start(rhs_hi_T[i][:], rhs_hi_d[:, i * WT:(i + 1) * WT])
                nc.gpsimd.dma_start(rhs_lo_T[i][:], rhs_lo_d[:, i * WT:(i + 1) * WT])

            cost_cols = out_pool.tile([128, NT], _F32, tag="cc")
            idx_cols = out_pool.tile([128, NT], _U32, tag="ic")
            cntu_cols = out_pool.tile([128, NT], _F32, tag="cu")
            cnta_cols = out_pool.tile([128, NT], _F32, tag="ca")
            cntb_cols = out_pool.tile([128, NT], _F32, tag="cb")
            cntc_cols = out_pool.tile([128, NT], _F32, tag="ccx")

            dump = dump_pool.tile([128, WT], _FP8, tag="dump")

            prev = None  # (x2, cma, cm1, cm2, m8, t) of previous tile

            def emit_counts(state):
                # small counts first: they unblock cm1/cm2 slab reuse
                x2_p, cma_p, cm1_p, cm2_p, m8_p, tp = state
                nc.scalar.activation(
                    dump[:, 0:W2F], cm1_p[:, W2F:WT],
                    mybir.ActivationFunctionType.Sign,
                    bias=m8_p[:, 0:1], scale=-1.0,
                    accum_out=cntb_cols[:, tp:tp + 1])
                nc.scalar.activation(
                    dump[:, 0:W3F], cm2_p[:, W3F:W2F],
                    mybir.ActivationFunctionType.Sign,
                    bias=m8_p[:, 0:1], scale=-1.0,
                    accum_out=cntc_cols[:, tp:tp + 1])
                nc.scalar.activation(
                    dump[:, 0:WT], x2_p[:],
                    mybir.ActivationFunctionType.Sign,
                    bias=m8_p[:, 0:1], scale=-1.0,
                    accum_out=cntu_cols[:, tp:tp + 1])
                nc.scalar.activation(
                    dump[:, 0:WT], cma_p[:],
                    mybir.ActivationFunctionType.Sign,
                    bias=m8_p[:, 0:1], scale=-1.0,
                    accum_out=cnta_cols[:, tp:tp + 1])

            for t in range(NT):
                lh = lhs_hi[:, t * 128:(t + 1) * 128]
                ll = lhs_lo[:, t * 128:(t + 1) * 128]

                x2 = x2_pool.tile([128, WT], _F32, tag="x2")
                cma = cma_pool.tile([128, WT], _F32, tag="cma")
                cm1 = cm1_pool.tile([128, WT], _F32, tag="cm1")

                GPT = WT // GRP  # groups per third
                # last third (U): evict to x2
                for k in range(GPT):
                    ps = ps_pool.tile([128, GRP], _F32, tag="ps")
                    sl = slice(k * GRP, (k + 1) * GRP)
                    _emit_group_mms(nc, ps[:], lh, ll, rhs_hi_T[2][:],
                                    rhs_lo_T[2][:], k * GRP)
                    nc.scalar.copy(x2[:, sl], ps[:])

                if prev is not None:
                    emit_counts(prev)

                # middle third: fold against x2 -> cma
                for k in range(GPT):
                    ps = ps_pool.tile([128, GRP], _F32, tag="ps")
                    sl = slice(k * GRP, (k + 1) * GRP)
                    _emit_group_mms(nc, ps[:], lh, ll, rhs_hi_T[1][:],
                                    rhs_lo_T[1][:], k * GRP)
                    nc.vector.tensor_max(cma[:, sl], ps[:], x2[:, sl])

                # first third: fold against cma -> cm1
                for k in range(GPT):
                    ps = ps_pool.tile([128, GRP], _F32, tag="ps")
                    sl = slice(k * GRP, (k + 1) * GRP)
                    _emit_group_mms(nc, ps[:], lh, ll, rhs_hi_T[0][:],
                                    rhs_lo_T[0][:], k * GRP)
                    nc.vector.tensor_max(cm1[:, sl], ps[:], cma[:, sl])

                cm2 = cm2_pool.tile([128, W2F], _F32, tag="cm2")
                nc.vector.tensor_max(cm2[:], cm1[:, 0:W2F], cm1[:, W2F:WT])
                cm3 = cm3_pool.tile([128, W3F], _F32, tag="cm3")
                nc.vector.tensor_max(cm3[:], cm2[:, 0:W3F], cm2[:, W3F:W2F])

                m8 = m8_pool.tile([128, 8], _F32, tag="m8")
                i8 = m8_pool.tile([128, 8], _U32, tag="i8")
                nc.vector.max(m8[:], cm3[:])
                nc.vector.max_index(i8[:], m8[:], cm3[:])
                nc.vector.tensor_copy(cost_cols[:, t:t + 1], m8[:, 0:1])
                nc.vector.tensor_copy(idx_cols[:, t:t + 1], i8[:, 0:1])

                prev = (x2, cma, cm1, cm2, m8, t)

            emit_counts(prev)

            nc.sync.dma_start(cost_d[:], cost_cols[:])
            nc.sync.dma_start(idx_d[:], idx_cols[:])
            nc.sync.dma_start(cntu_d[:], cntu_cols[:])
            nc.sync.dma_start(cnta_d[:], cnta_cols[:])
            nc.sync.dma_start(cntb_d[:], cntb_cols[:])
            nc.sync.dma_start(cntc_d[:], cntc_cols[:])

    nc.finalize()
    return nc


def _get_nc():
    if "nc" not in _compiled:
        _compiled["nc"] = _build_nc()
    return _compiled["nc"]


def _split_bf16(x):
    hi = x.astype(ml_dtypes.bfloat16)
    lo = (x - hi.astype(np.float32)).astype(ml_dtypes.bfloat16)
    return hi, lo


def _make_in_maps(feat_l, feat_r):
    rhs = np.ascontiguousarray(feat_r.reshape(C, NPIX))
    rhs_hi, rhs_lo = _split_bf16(rhs)
    in_maps = []
    for k in range(NCORES):
        lhs = np.ascontiguousarray(
            feat_l[0, :, k * ROWS_PER_CORE:(k + 1) * ROWS_PER_CORE, :]
            .reshape(C, ROWS_PER_CORE * W))
        lhs_hi, lhs_lo = _split_bf16(lhs)
        in_maps.append({
            "lhs_hi": lhs_hi, "lhs_lo": lhs_lo,
            "rhs_hi": rhs_hi, "rhs_lo": rhs_lo,
        })
    return in_maps


def _decode(results, scale_x, scale_y):
    flow_cost = np.empty((B, H, W), np.float32)
    flat = np.empty((H, W), np.int64)
    for k, r in enumerate(results):
        rows = slice(k * ROWS_PER_CORE, (k + 1) * ROWS_PER_CORE)
        flow_cost[0, rows, :] = r["cost"].T
        aU = (WT - r["cntu"].T.astype(np.int64))
        aA = (WT - r["cnta"].T.astype(np.int64))
        b = (W2F - r["cntb"].T.astype(np.int64))
        c = (W3F - r["cntc"].T.astype(np.int64))
        i_res = r["idx"].T.astype(np.int64)
        base = np.where(aU > 0, 2 * WT, np.where(aA > 0, WT, 0))
        flat[rows, :] = i_res + W3F * c + W2F * b + base
    np.clip(flat, 0, NPIX - 1, out=flat)
    xoff = np.arange(W)
    yoff = np.arange(H)
    u = -((flat % W_R) * scale_x - xoff[None, :]).astype(np.float32)
    v = -((flat // W_R) * scale_y - yoff[:, None]).astype(np.float32)
    flow = np.stack([u, v], axis=2)[None]  # [1, H, W, 2]
    return flow, flow_cost


def kernel(feat_l, feat_r, scale_x, scale_y):
    feat_l = np.asarray(feat_l, dtype=np.float32)
    feat_r = np.asarray(feat_r, dtype=np.float32)
    nc = _get_nc()
    in_maps = _make_in_maps(feat_l, feat_r)
    res = run_bass_kernel_spmd(nc, in_maps, core_ids=list(range(NCORES)))
    return _decode(res.results, int(scale_x), int(scale_y))


def run_timed(np_inputs, trace_cores=None):
    """Run once with NTFF tracing enabled; returns exec_time_ns (or None)."""
    feat_l = np.asarray(np_inputs["feat_l"], dtype=np.float32)
    feat_r = np.asarray(np_inputs["feat_r"], dtype=np.float32)
    nc = _get_nc()
    in_maps = _make_in_maps(feat_l, feat_r)
    res = run_bass_kernel_spmd(nc, in_maps, core_ids=list(range(NCORES)),
                               trace=True, trace_cores=trace_cores)
    if res.instructions_and_trace is not None:
        print("trace path:", res.instructions_and_trace[1])
    return res.exec_time_ns


# revision 19
# speedup vs baseline: 1.0028x; 1.0028x over previous
"""Distributed volume-argmin (correlation volume max/argmax) on 8 NeuronCores.

Problem: feat_l, feat_r [1, 128, 96, 128] fp32.
  corr[b,h,w,ij] = sum_c feat_l[b,c,h,w] * feat_r[b,c,i,j]
  flow_cost = max over ij, flat = argmax over ij,
  flow = (xoff - (flat % W_R)*scale_x, yoff - (flat // W_R)*scale_y)

Sharding: the 96 left-image rows are split across 8 cores (12 rows = 1536
left pixels each); feat_r is replicated. Each core computes its
[1536 x 12288] correlation block (K=C=128 contraction on partitions) and a
full per-pixel max/argmax, so no cross-core reduction is needed — outputs
concatenate.

Matmul precision: inputs are split host-side into bf16 hi/lo pairs and each
512-chunk accumulates hi*hi + hi*lo + lo*hi in fp32 PSUM (fp32-grade
accuracy at 3x-bf16-pass cost).

Argmax strategy per 128-pixel tile (partition = left pixel, free = 12288
right pixels): a ternary-then-binary fold tree on the Vector engine.
The 12288 row is split in thirds T1|T2|U; U (groups 4,5) is evicted
PSUM->SBUF by ScalarE (x2), then
  cma[j] = max(T2[j] (PSUM), x2[j])         j in [0,4096)
  cm1[j] = max(T1[j] (PSUM), cma[j])
  cm2[j] = max(cm1[j], cm1[j+2048])
  cm3[j] = max(cm2[j], cm2[j+1024])
so the DVE folds double as the PSUM drain. MAX8 + FIND_INDEX8 on the
1024-wide cm3 give the exact max M and the fold residue i*. The fold
branch bits are recovered exactly on the Scalar engine with its free
sum-accumulator: count = sum_j sign(M - y[j]) over y in {x2, cma,
cm1_hi, cm2_hi} equals the array size minus 1 exactly when M lives in
that array (sign(0)=0 on HW, verified); flat = base + 2048*b + 1024*c
+ i* with base = 8192 if M in U else 4096 if M in T2|U else 0. All
values are exact fp32 throughout, and ties resolve like jnp.argmax
(first occurrence) because corr maxima are unique for continuous data.
"""

import sys

for p in ("/opt/trn_rl_repo",):
    if p not in sys.path:
        sys.path.insert(0, p)

import numpy as np
import ml_dtypes

import concourse.bass as bass
import concourse.tile as tile
from concourse import bacc, mybir
from concourse.bass_utils import run_bass_kernel_spmd

# Problem geometry (hardcoded per the task contract).
B, C, H, W = 1, 128, 96, 128
H_R, W_R = 96, 128
NPIX = H_R * W_R              # 12288 right pixels
NCORES = 8
ROWS_PER_CORE = H // NCORES   # 12 left rows per core
NT = ROWS_PER_CORE            # one tile per left row (128 pixels)
CHUNK = 512
GRP = 1024                    # psum group (2 banks)
NGRP = NPIX // GRP            # 6
WT = NPIX // 3               # third width: 4096 (ternary first level)
W2F = WT // 2                 # 2048 (fold2 output)
W3F = WT // 4                 # 1024 (fold3 output)

_F32 = mybir.dt.float32
_BF16 = mybir.dt.bfloat16
_U32 = mybir.dt.uint32
_FP8 = mybir.dt.float8e4

_compiled = {}


def _emit_group_mms(nc, ps_ap, lh, ll, rh_t, rl_t, local_base):
    """3-pass bf16 matmuls for the chunks of one psum group.

    Pass-major order (hi*hi, hi*lo, lo*hi across chunks) so consecutive
    matmuls write different PSUM banks — same-bank accumulation
    back-to-back serializes the PE's fill/drain pipeline (379 vs 216 ns).
    rh_t/rl_t are per-third rhs tiles; local_base is the column offset of
    this group within its third."""
    for pass_lhs, pass_rhs, st, sp in (
        (lh, rh_t, True, False),
        (lh, rl_t, False, False),
        (ll, rh_t, False, True),
    ):
        for s in range(GRP // CHUNK):
            col0 = local_base + s * CHUNK
            out = ps_ap[:, s * CHUNK:(s + 1) * CHUNK]
            nc.tensor.matmul(out, pass_lhs, pass_rhs[:, col0:col0 + CHUNK],
                             start=st, stop=sp)


def _build_nc():
    nc = bacc.Bacc("TRN2", target_bir_lowering=False, debug=False,
                   num_devices=NCORES)

    lhs_hi_d = nc.declare_dram_parameter("lhs_hi", [C, NT * 128], _BF16,
                                         isOutput=False)
    lhs_lo_d = nc.declare_dram_parameter("lhs_lo", [C, NT * 128], _BF16,
                                         isOutput=False)
    rhs_hi_d = nc.declare_dram_parameter("rhs_hi", [C, NPIX], _BF16,
                                         isOutput=False)
    rhs_lo_d = nc.declare_dram_parameter("rhs_lo", [C, NPIX], _BF16,
                                         isOutput=False)
    cost_d = nc.declare_dram_parameter("cost", [128, NT], _F32, isOutput=True)
    idx_d = nc.declare_dram_parameter("idx", [128, NT], _U32, isOutput=True)
    cntu_d = nc.declare_dram_parameter("cntu", [128, NT], _F32, isOutput=True)
    cnta_d = nc.declare_dram_parameter("cnta", [128, NT], _F32, isOutput=True)
    cntb_d = nc.declare_dram_parameter("cntb", [128, NT], _F32, isOutput=True)
    cntc_d = nc.declare_dram_parameter("cntc", [128, NT], _F32, isOutput=True)

    with tile.TileContext(nc) as tc:
        with (
            tc.tile_pool(name="rhs", bufs=1) as rhs_pool,
            tc.tile_pool(name="lhs", bufs=1) as lhs_pool,
            tc.tile_pool(name="x2p", bufs=2) as x2_pool,
            tc.tile_pool(name="cmap", bufs=2) as cma_pool,
            tc.tile_pool(name="cm1p", bufs=2) as cm1_pool,
            tc.tile_pool(name="cm2p", bufs=1) as cm2_pool,
            tc.tile_pool(name="cm3p", bufs=2) as cm3_pool,
            tc.tile_pool(name="dumpp", bufs=1) as dump_pool,
            tc.tile_pool(name="m8p", bufs=3) as m8_pool,
            tc.tile_pool(name="ps", bufs=4, space="PSUM") as ps_pool,
            tc.tile_pool(name="outcols", bufs=1) as out_pool,
        ):
            lhs_hi = lhs_pool.tile([C, NT * 128], _BF16, tag="lh")
            lhs_lo = lhs_pool.tile([C, NT * 128], _BF16, tag="ll")
            # per-third rhs tiles: Tile tracks DMA dependencies per tile, so
            # the first matmuls wait only for their own third's load instead
            # of the whole 6.3 MB rhs. hi via sync (HWDGE), lo via gpsimd
            # (SWDGE). Load order: U third (needed first), then T2, then T1.
            rh0 = rhs_pool.tile([C, WT], _BF16, tag="rh0")
            rh1 = rhs_pool.tile([C, WT], _BF16, tag="rh1")
            rl0 = rhs_pool.tile([C, WT], _BF16, tag="rl0")
            rl1 = rhs_pool.tile([C, WT], _BF16, tag="rl1")
            rhs_hi_T = [rh0, rh1]
            rhs_lo_T = [rl0, rl1]
            # the U third (needed first by every tile) is split into
            # per-group tiles so the very first matmuls wait only for a
            # 0.26 MB load, not the whole third
            rhu0 = rhs_pool.tile([C, GRP], _BF16, tag="rhu0")
            rhu1 = rhs_pool.tile([C, GRP], _BF16, tag="rhu1")
            rhu2 = rhs_pool.tile([C, GRP], _BF16, tag="rhu2")
            rhu3 = rhs_pool.tile([C, GRP], _BF16, tag="rhu3")
            rlu0 = rhs_pool.tile([C, GRP], _BF16, tag="rlu0")
            rlu1 = rhs_pool.tile([C, GRP], _BF16, tag="rlu1")
            rlu2 = rhs_pool.tile([C, GRP], _BF16, tag="rlu2")
            rlu3 = rhs_pool.tile([C, GRP], _BF16, tag="rlu3")
            rhs_hi_U = [rhu0, rhu1, rhu2, rhu3]
            rhs_lo_U = [rlu0, rlu1, rlu2, rlu3]
            u0 = 2 * WT
            nc.sync.dma_start(rhs_hi_U[0][:], rhs_hi_d[:, u0:u0 + GRP])
            nc.sync.dma_start(lhs_hi[:], lhs_hi_d[:])
            nc.sync.dma_start(rhs_lo_U[0][:], rhs_lo_d[:, u0:u0 + GRP])
            nc.sync.dma_start(lhs_lo[:], lhs_lo_d[:])
            for k in range(1, 4):
                sl = slice(u0 + k * GRP, u0 + (k + 1) * GRP)
                nc.sync.dma_start(rhs_hi_U[k][:], rhs_hi_d[:, sl])
                nc.gpsimd.dma_start(rhs_lo_U[k][:], rhs_lo_d[:, sl])
            for i in (1, 0):
                nc.sync.dma_start(rhs_hi_T[i][:], rhs_hi_d[:, i * WT:(i + 1) * WT])
                nc.gpsimd.dma_start(rhs_lo_T[i][:], rhs_lo_d[:, i * WT:(i + 1) * WT])

            cost_cols = out_pool.tile([128, NT], _F32, tag="cc")
            idx_cols = out_pool.tile([128, NT], _U32, tag="ic")
            cntu_cols = out_pool.tile([128, NT], _F32, tag="cu")
            cnta_cols = out_pool.tile([128, NT], _F32, tag="ca")
            cntb_cols = out_pool.tile([128, NT], _F32, tag="cb")
            cntc_cols = out_pool.tile([128, NT], _F32, tag="ccx")

            dump = dump_pool.tile([128, WT], _FP8, tag="dump")

            prev = None  # (x2, cma, cm1, cm2, m8, t) of previous tile

            def emit_counts(state):
                # small counts first: they unblock cm1/cm2 slab reuse
                x2_p, cma_p, cm1_p, cm2_p, m8_p, tp = state
                nc.scalar.activation(
                    dump[:, 0:W2F], cm1_p[:, W2F:WT],
                    mybir.ActivationFunctionType.Sign,
                    bias=m8_p[:, 0:1], scale=-1.0,
                    accum_out=cntb_cols[:, tp:tp + 1])
                nc.scalar.activation(
                    dump[:, 0:W3F], cm2_p[:, W3F:W2F],
                    mybir.ActivationFunctionType.Sign,
                    bias=m8_p[:, 0:1], scale=-1.0,
                    accum_out=cntc_cols[:, tp:tp + 1])
                nc.scalar.activation(
                    dump[:, 0:WT], x2_p[:],
                    mybir.ActivationFunctionType.Sign,
                    bias=m8_p[:, 0:1], scale=-1.0,
                    accum_out=cntu_cols[:, tp:tp + 1])
                nc.scalar.activation(
                    dump[:, 0:WT], cma_p[:],
                    mybir.ActivationFunctionType.Sign,
                    bias=m8_p[:, 0:1], scale=-1.0,
                    accum_out=cnta_cols[:, tp:tp + 1])

            for t in range(NT):
                lh = lhs_hi[:, t * 128:(t + 1) * 128]
                ll = lhs_lo[:, t * 128:(t + 1) * 128]

                x2 = x2_pool.tile([128, WT], _F32, tag="x2")
                cma = cma_pool.tile([128, WT], _F32, tag="cma")
                cm1 = cm1_pool.tile([128, WT], _F32, tag="cm1")

                GPT = WT // GRP  # groups per third
                # last third (U): evict to x2
                for k in range(GPT):
                    ps = ps_pool.tile([128, GRP], _F32, tag="ps")
                    sl = slice(k * GRP, (k + 1) * GRP)
                    _emit_group_mms(nc, ps[:], lh, ll, rhs_hi_U[k][:],
                                    rhs_lo_U[k][:], 0)
                    nc.scalar.copy(x2[:, sl], ps[:])

                if prev is not None:
                    emit_counts(prev)

                # middle third: fold against x2 -> cma
                for k in range(GPT):
                    ps = ps_pool.tile([128, GRP], _F32, tag="ps")
                    sl = slice(k * GRP, (k + 1) * GRP)
                    _emit_group_mms(nc, ps[:], lh, ll, rhs_hi_T[1][:],
                                    rhs_lo_T[1][:], k * GRP)
                    nc.vector.tensor_max(cma[:, sl], ps[:], x2[:, sl])

                # first third: fold against cma -> cm1
                for k in range(GPT):
                    ps = ps_pool.tile([128, GRP], _F32, tag="ps")
                    sl = slice(k * GRP, (k + 1) * GRP)
                    _emit_group_mms(nc, ps[:], lh, ll, rhs_hi_T[0][:],
                                    rhs_lo_T[0][:], k * GRP)
                    nc.vector.tensor_max(cm1[:, sl], ps[:], cma[:, sl])

                cm2 = cm2_pool.tile([128, W2F], _F32, tag="cm2")
                nc.vector.tensor_max(cm2[:], cm1[:, 0:W2F], cm1[:, W2F:WT])
                cm3 = cm3_pool.tile([128, W3F], _F32, tag="cm3")
                nc.vector.tensor_max(cm3[:], cm2[:, 0:W3F], cm2[:, W3F:W2F])

                m8 = m8_pool.tile([128, 8], _F32, tag="m8")
                i8 = m8_pool.tile([128, 8], _U32, tag="i8")
                nc.vector.max(m8[:], cm3[:])
                nc.vector.max_index(i8[:], m8[:], cm3[:])
                nc.vector.tensor_copy(cost_cols[:, t:t + 1], m8[:, 0:1])
                nc.vector.tensor_copy(idx_cols[:, t:t + 1], i8[:, 0:1])

                prev = (x2, cma, cm1, cm2, m8, t)

            emit_counts(prev)

            nc.sync.dma_start(cost_d[:], cost_cols[:])
            nc.sync.dma_start(idx_d[:], idx_cols[:])
            nc.sync.dma_start(cntu_d[:], cntu_cols[:])
            nc.sync.dma_start(cnta_d[:], cnta_cols[:])
            nc.sync.dma_start(cntb_d[:], cntb_cols[:])
            nc.sync.dma_start(cntc_d[:], cntc_cols[:])

    nc.finalize()
    return nc


def _get_nc():
    if "nc" not in _compiled:
        _compiled["nc"] = _build_nc()
    return _compiled["nc"]


def _split_bf16(x):
    hi = x.astype(ml_dtypes.bfloat16)
    lo = (x - hi.astype(np.float32)).astype(ml_dtypes.bfloat16)
    return hi, lo


def _make_in_maps(feat_l, feat_r):
    rhs = np.ascontiguousarray(feat_r.reshape(C, NPIX))
    rhs_hi, rhs_lo = _split_bf16(rhs)
    in_maps = []
    for k in range(NCORES):
        lhs = np.ascontiguousarray(
            feat_l[0, :, k * ROWS_PER_CORE:(k + 1) * ROWS_PER_CORE, :]
            .reshape(C, ROWS_PER_CORE * W))
        lhs_hi, lhs_lo = _split_bf16(lhs)
        in_maps.append({
            "lhs_hi": lhs_hi, "lhs_lo": lhs_lo,
            "rhs_hi": rhs_hi, "rhs_lo": rhs_lo,
        })
    return in_maps


def _decode(results, scale_x, scale_y):
    flow_cost = np.empty((B, H, W), np.float32)
    flat = np.empty((H, W), np.int64)
    for k, r in enumerate(results):
        rows = slice(k * ROWS_PER_CORE, (k + 1) * ROWS_PER_CORE)
        flow_cost[0, rows, :] = r["cost"].T
        aU = (WT - r["cntu"].T.astype(np.int64))
        aA = (WT - r["cnta"].T.astype(np.int64))
        b = (W2F - r["cntb"].T.astype(np.int64))
        c = (W3F - r["cntc"].T.astype(np.int64))
        i_res = r["idx"].T.astype(np.int64)
        base = np.where(aU > 0, 2 * WT, np.where(aA > 0, WT, 0))
        flat[rows, :] = i_res + W3F * c + W2F * b + base
    np.clip(flat, 0, NPIX - 1, out=flat)
    xoff = np.arange(W)
    yoff = np.arange(H)
    u = -((flat % W_R) * scale_x - xoff[None, :]).astype(np.float32)
    v = -((flat // W_R) * scale_y - yoff[:, None]).astype(np.float32)
    flow = np.stack([u, v], axis=2)[None]  # [1, H, W, 2]
    return flow, flow_cost


def kernel(feat_l, feat_r, scale_x, scale_y):
    feat_l = np.asarray(feat_l, dtype=np.float32)
    feat_r = np.asarray(feat_r, dtype=np.float32)
    nc = _get_nc()
    in_maps = _make_in_maps(feat_l, feat_r)
    res = run_bass_kernel_spmd(nc, in_maps, core_ids=list(range(NCORES)))
    return _decode(res.results, int(scale_x), int(scale_y))


def run_timed(np_inputs, trace_cores=None):
    """Run once with NTFF tracing enabled; returns exec_time_ns (or None)."""
    feat_l = np.asarray(np_inputs["feat_l"], dtype=np.float32)
    feat_r = np.asarray(np_inputs["feat_r"], dtype=np.float32)
    nc = _get_nc()
    in_maps = _make_in_maps(feat_l, feat_r)
    res = run_bass_kernel_spmd(nc, in_maps, core_ids=list(range(NCORES)),
                               trace=True, trace_cores=trace_cores)
    if res.instructions_and_trace is not None:
        print("trace path:", res.instructions_and_trace[1])
    return res.exec_time_ns


# revision 21
# speedup vs baseline: 1.0036x; 1.0008x over previous
"""Distributed volume-argmin (correlation volume max/argmax) on 8 NeuronCores.

Problem: feat_l, feat_r [1, 128, 96, 128] fp32.
  corr[b,h,w,ij] = sum_c feat_l[b,c,h,w] * feat_r[b,c,i,j]
  flow_cost = max over ij, flat = argmax over ij,
  flow = (xoff - (flat % W_R)*scale_x, yoff - (flat // W_R)*scale_y)

Sharding: the 96 left-image rows are split across 8 cores (12 rows = 1536
left pixels each); feat_r is replicated. Each core computes its
[1536 x 12288] correlation block (K=C=128 contraction on partitions) and a
full per-pixel max/argmax, so no cross-core reduction is needed — outputs
concatenate.

Matmul precision: inputs are split host-side into bf16 hi/lo pairs and each
512-chunk accumulates hi*hi + hi*lo + lo*hi in fp32 PSUM (fp32-grade
accuracy at 3x-bf16-pass cost).

Argmax strategy per 128-pixel tile (partition = left pixel, free = 12288
right pixels): a ternary-then-binary fold tree on the Vector engine.
The 12288 row is split in thirds T1|T2|U; U (groups 4,5) is evicted
PSUM->SBUF by ScalarE (x2), then
  cma[j] = max(T2[j] (PSUM), x2[j])         j in [0,4096)
  cm1[j] = max(T1[j] (PSUM), cma[j])
  cm2[j] = max(cm1[j], cm1[j+2048])
  cm3[j] = max(cm2[j], cm2[j+1024])
so the DVE folds double as the PSUM drain. MAX8 + FIND_INDEX8 on the
1024-wide cm3 give the exact max M and the fold residue i*. The fold
branch bits are recovered exactly on the Scalar engine with its free
sum-accumulator: count = sum_j sign(M - y[j]) over y in {x2, cma,
cm1_hi, cm2_hi} equals the array size minus 1 exactly when M lives in
that array (sign(0)=0 on HW, verified); flat = base + 2048*b + 1024*c
+ i* with base = 8192 if M in U else 4096 if M in T2|U else 0. All
values are exact fp32 throughout, and ties resolve like jnp.argmax
(first occurrence) because corr maxima are unique for continuous data.
"""

import sys

for p in ("/opt/trn_rl_repo",):
    if p not in sys.path:
        sys.path.insert(0, p)

import numpy as np
import ml_dtypes

import concourse.bass as bass
import concourse.tile as tile
from concourse import bacc, mybir
from concourse.bass_utils import run_bass_kernel_spmd

# Problem geometry (hardcoded per the task contract).
B, C, H, W = 1, 128, 96, 128
H_R, W_R = 96, 128
NPIX = H_R * W_R              # 12288 right pixels
NCORES = 8
ROWS_PER_CORE = H // NCORES   # 12 left rows per core
NT = ROWS_PER_CORE            # one tile per left row (128 pixels)
CHUNK = 512
GRP = 1024                    # psum group (2 banks)
NGRP = NPIX // GRP            # 6
WT = NPIX // 3               # third width: 4096 (ternary first level)
W2F = WT // 2                 # 2048 (fold2 output)
W3F = WT // 4                 # 1024 (fold3 output)

_F32 = mybir.dt.float32
_BF16 = mybir.dt.bfloat16
_U32 = mybir.dt.uint32
_FP8 = mybir.dt.float8e4

_compiled = {}


def _emit_group_mms(nc, ps_ap, lh, ll, rh_t, rl_t, local_base):
    """3-pass bf16 matmuls for the chunks of one psum group.

    Pass-major order (hi*hi, hi*lo, lo*hi across chunks) so consecutive
    matmuls write different PSUM banks — same-bank accumulation
    back-to-back serializes the PE's fill/drain pipeline (379 vs 216 ns).
    rh_t/rl_t are per-third rhs tiles; local_base is the column offset of
    this group within its third."""
    for pass_lhs, pass_rhs, st, sp in (
        (lh, rh_t, True, False),
        (lh, rl_t, False, False),
        (ll, rh_t, False, True),
    ):
        for s in range(GRP // CHUNK):
            col0 = local_base + s * CHUNK
            out = ps_ap[:, s * CHUNK:(s + 1) * CHUNK]
            nc.tensor.matmul(out, pass_lhs, pass_rhs[:, col0:col0 + CHUNK],
                             start=st, stop=sp)


def _build_nc():
    nc = bacc.Bacc("TRN2", target_bir_lowering=False, debug=False,
                   num_devices=NCORES)

    lhs_hi_d = nc.declare_dram_parameter("lhs_hi", [C, NT * 128], _BF16,
                                         isOutput=False)
    lhs_lo_d = nc.declare_dram_parameter("lhs_lo", [C, NT * 128], _BF16,
                                         isOutput=False)
    rhs_hi_d = nc.declare_dram_parameter("rhs_hi", [C, NPIX], _BF16,
                                         isOutput=False)
    rhs_lo_d = nc.declare_dram_parameter("rhs_lo", [C, NPIX], _BF16,
                                         isOutput=False)
    cost_d = nc.declare_dram_parameter("cost", [128, NT], _F32, isOutput=True)
    idx_d = nc.declare_dram_parameter("idx", [128, NT], _U32, isOutput=True)
    cntu_d = nc.declare_dram_parameter("cntu", [128, NT], _F32, isOutput=True)
    cnta_d = nc.declare_dram_parameter("cnta", [128, NT], _F32, isOutput=True)
    cntb_d = nc.declare_dram_parameter("cntb", [128, NT], _F32, isOutput=True)
    cntc_d = nc.declare_dram_parameter("cntc", [128, NT], _F32, isOutput=True)

    with tile.TileContext(nc) as tc:
        with (
            tc.tile_pool(name="rhs", bufs=1) as rhs_pool,
            tc.tile_pool(name="lhs", bufs=1) as lhs_pool,
            tc.tile_pool(name="x2p", bufs=2) as x2_pool,
            tc.tile_pool(name="cmap", bufs=2) as cma_pool,
            tc.tile_pool(name="cm1p", bufs=2) as cm1_pool,
            tc.tile_pool(name="cm2p", bufs=1) as cm2_pool,
            tc.tile_pool(name="cm3p", bufs=2) as cm3_pool,
            tc.tile_pool(name="dumpp", bufs=1) as dump_pool,
            tc.tile_pool(name="m8p", bufs=3) as m8_pool,
            tc.tile_pool(name="ps", bufs=4, space="PSUM") as ps_pool,
            tc.tile_pool(name="outcols", bufs=1) as out_pool,
        ):
            # tile 0's lhs columns split out so the first matmuls gate on
            # a 33 KB load instead of the full 0.4 MB lhs tensor
            lhs_hi0 = lhs_pool.tile([C, 128], _BF16, tag="lh0")
            lhs_lo0 = lhs_pool.tile([C, 128], _BF16, tag="ll0")
            lhs_hi = lhs_pool.tile([C, NT * 128], _BF16, tag="lh")
            lhs_lo = lhs_pool.tile([C, NT * 128], _BF16, tag="ll")
            # per-third rhs tiles: Tile tracks DMA dependencies per tile, so
            # the first matmuls wait only for their own third's load instead
            # of the whole 6.3 MB rhs. hi via sync (HWDGE), lo via gpsimd
            # (SWDGE). Load order: U third (needed first), then T2, then T1.
            rh0 = rhs_pool.tile([C, WT], _BF16, tag="rh0")
            rh1 = rhs_pool.tile([C, WT], _BF16, tag="rh1")
            rh2 = rhs_pool.tile([C, WT], _BF16, tag="rh2")
            rl0 = rhs_pool.tile([C, WT], _BF16, tag="rl0")
            rl1 = rhs_pool.tile([C, WT], _BF16, tag="rl1")
            rl2 = rhs_pool.tile([C, WT], _BF16, tag="rl2")
            rhs_hi_T = [rh0, rh1, rh2]
            rhs_lo_T = [rl0, rl1, rl2]
            nc.sync.dma_start(lhs_hi0[:], lhs_hi_d[:, 0:128])
            nc.sync.dma_start(rhs_hi_T[2][:], rhs_hi_d[:, 2 * WT:3 * WT])
            nc.sync.dma_start(lhs_lo0[:], lhs_lo_d[:, 0:128])
            nc.gpsimd.dma_start(rhs_lo_T[2][:], rhs_lo_d[:, 2 * WT:3 * WT])
            nc.sync.dma_start(lhs_hi[:], lhs_hi_d[:])
            nc.gpsimd.dma_start(lhs_lo[:], lhs_lo_d[:])
            for i in (1, 0):
                nc.sync.dma_start(rhs_hi_T[i][:], rhs_hi_d[:, i * WT:(i + 1) * WT])
                nc.gpsimd.dma_start(rhs_lo_T[i][:], rhs_lo_d[:, i * WT:(i + 1) * WT])

            cost_cols = out_pool.tile([128, NT], _F32, tag="cc")
            idx_cols = out_pool.tile([128, NT], _U32, tag="ic")
            cntu_cols = out_pool.tile([128, NT], _F32, tag="cu")
            cnta_cols = out_pool.tile([128, NT], _F32, tag="ca")
            cntb_cols = out_pool.tile([128, NT], _F32, tag="cb")
            cntc_cols = out_pool.tile([128, NT], _F32, tag="ccx")

            dump = dump_pool.tile([128, WT], _FP8, tag="dump")

            prev = None  # (x2, cma, cm1, cm2, m8, t) of previous tile

            def emit_counts(state):
                # small counts first: they unblock cm1/cm2 slab reuse
                x2_p, cma_p, cm1_p, cm2_p, m8_p, tp = state
                nc.scalar.activation(
                    dump[:, 0:W2F], cm1_p[:, W2F:WT],
                    mybir.ActivationFunctionType.Sign,
                    bias=m8_p[:, 0:1], scale=-1.0,
                    accum_out=cntb_cols[:, tp:tp + 1])
                nc.scalar.activation(
                    dump[:, 0:W3F], cm2_p[:, W3F:W2F],
                    mybir.ActivationFunctionType.Sign,
                    bias=m8_p[:, 0:1], scale=-1.0,
                    accum_out=cntc_cols[:, tp:tp + 1])
                nc.scalar.activation(
                    dump[:, 0:WT], x2_p[:],
                    mybir.ActivationFunctionType.Sign,
                    bias=m8_p[:, 0:1], scale=-1.0,
                    accum_out=cntu_cols[:, tp:tp + 1])
                nc.scalar.activation(
                    dump[:, 0:WT], cma_p[:],
                    mybir.ActivationFunctionType.Sign,
                    bias=m8_p[:, 0:1], scale=-1.0,
                    accum_out=cnta_cols[:, tp:tp + 1])

            for t in range(NT):
                if t == 0:
                    lh, ll = lhs_hi0[:], lhs_lo0[:]
                else:
                    lh = lhs_hi[:, t * 128:(t + 1) * 128]
                    ll = lhs_lo[:, t * 128:(t + 1) * 128]

                x2 = x2_pool.tile([128, WT], _F32, tag="x2")
                cma = cma_pool.tile([128, WT], _F32, tag="cma")
                cm1 = cm1_pool.tile([128, WT], _F32, tag="cm1")

                GPT = WT // GRP  # groups per third
                # last third (U): evict to x2
                for k in range(GPT):
                    ps = ps_pool.tile([128, GRP], _F32, tag="ps")
                    sl = slice(k * GRP, (k + 1) * GRP)
                    _emit_group_mms(nc, ps[:], lh, ll, rhs_hi_T[2][:],
                                    rhs_lo_T[2][:], k * GRP)
                    nc.scalar.copy(x2[:, sl], ps[:])

                if prev is not None:
                    emit_counts(prev)

                # middle third: fold against x2 -> cma
                for k in range(GPT):
                    ps = ps_pool.tile([128, GRP], _F32, tag="ps")
                    sl = slice(k * GRP, (k + 1) * GRP)
                    _emit_group_mms(nc, ps[:], lh, ll, rhs_hi_T[1][:],
                                    rhs_lo_T[1][:], k * GRP)
                    nc.vector.tensor_max(cma[:, sl], ps[:], x2[:, sl])

                # first third: fold against cma -> cm1
                for k in range(GPT):
                    ps = ps_pool.tile([128, GRP], _F32, tag="ps")
                    sl = slice(k * GRP, (k + 1) * GRP)
                    _emit_group_mms(nc, ps[:], lh, ll, rhs_hi_T[0][:],
                                    rhs_lo_T[0][:], k * GRP)
                    nc.vector.tensor_max(cm1[:, sl], ps[:], cma[:, sl])

                cm2 = cm2_pool.tile([128, W2F], _F32, tag="cm2")
                nc.vector.tensor_max(cm2[:], cm1[:, 0:W2F], cm1[:, W2F:WT])
                cm3 = cm3_pool.tile([128, W3F], _F32, tag="cm3")
                nc.vector.tensor_max(cm3[:], cm2[:, 0:W3F], cm2[:, W3F:W2F])

                m8 = m8_pool.tile([128, 8], _F32, tag="m8")
                i8 = m8_pool.tile([128, 8], _U32, tag="i8")
                nc.vector.max(m8[:], cm3[:])
                nc.vector.max_index(i8[:], m8[:], cm3[:])
                nc.vector.tensor_copy(cost_cols[:, t:t + 1], m8[:, 0:1])
                nc.vector.tensor_copy(idx_cols[:, t:t + 1], i8[:, 0:1])

                prev = (x2, cma, cm1, cm2, m8, t)

            emit_counts(prev)

            nc.sync.dma_start(cost_d[:], cost_cols[:])
            nc.sync.dma_start(idx_d[:], idx_cols[:])
            nc.sync.dma_start(cntu_d[:], cntu_cols[:])
            nc.sync.dma_start(cnta_d[:], cnta_cols[:])
            nc.sync.dma_start(cntb_d[:], cntb_cols[:])
            nc.sync.dma_start(cntc_d[:], cntc_cols[:])

    nc.finalize()
    return nc


def _get_nc():
    if "nc" not in _compiled:
        _compiled["nc"] = _build_nc()
    return _compiled["nc"]


def _split_bf16(x):
    hi = x.astype(ml_dtypes.bfloat16)
    lo = (x - hi.astype(np.float32)).astype(ml_dtypes.bfloat16)
    return hi, lo


def _make_in_maps(feat_l, feat_r):
    rhs = np.ascontiguousarray(feat_r.reshape(C, NPIX))
    rhs_hi, rhs_lo = _split_bf16(rhs)
    in_maps = []
    for k in range(NCORES):
        lhs = np.ascontiguousarray(
            feat_l[0, :, k * ROWS_PER_CORE:(k + 1) * ROWS_PER_CORE, :]
            .reshape(C, ROWS_PER_CORE * W))
        lhs_hi, lhs_lo = _split_bf16(lhs)
        in_maps.append({
            "lhs_hi": lhs_hi, "lhs_lo": lhs_lo,
            "rhs_hi": rhs_hi, "rhs_lo": rhs_lo,
        })
    return in_maps


def _decode(results, scale_x, scale_y):
    flow_cost = np.empty((B, H, W), np.float32)
    flat = np.empty((H, W), np.int64)
    for k, r in enumerate(results):
        rows = slice(k * ROWS_PER_CORE, (k + 1) * ROWS_PER_CORE)
        flow_cost[0, rows, :] = r["cost"].T
        aU = (WT - r["cntu"].T.astype(np.int64))
        aA = (WT - r["cnta"].T.astype(np.int64))
        b = (W2F - r["cntb"].T.astype(np.int64))
        c = (W3F - r["cntc"].T.astype(np.int64))
        i_res = r["idx"].T.astype(np.int64)
        base = np.where(aU > 0, 2 * WT, np.where(aA > 0, WT, 0))
        flat[rows, :] = i_res + W3F * c + W2F * b + base
    np.clip(flat, 0, NPIX - 1, out=flat)
    xoff = np.arange(W)
    yoff = np.arange(H)
    u = -((flat % W_R) * scale_x - xoff[None, :]).astype(np.float32)
    v = -((flat // W_R) * scale_y - yoff[:, None]).astype(np.float32)
    flow = np.stack([u, v], axis=2)[None]  # [1, H, W, 2]
    return flow, flow_cost


def kernel(feat_l, feat_r, scale_x, scale_y):
    feat_l = np.asarray(feat_l, dtype=np.float32)
    feat_r = np.asarray(feat_r, dtype=np.float32)
    nc = _get_nc()
    in_maps = _make_in_maps(feat_l, feat_r)
    res = run_bass_kernel_spmd(nc, in_maps, core_ids=list(range(NCORES)))
    return _decode(res.results, int(scale_x), int(scale_y))


def run_timed(np_inputs, trace_cores=None):
    """Run once with NTFF tracing enabled; returns exec_time_ns (or None)."""
    feat_l = np.asarray(np_inputs["feat_l"], dtype=np.float32)
    feat_r = np.asarray(np_inputs["feat_r"], dtype=np.float32)
    nc = _get_nc()
    in_maps = _make_in_maps(feat_l, feat_r)
    res = run_bass_kernel_spmd(nc, in_maps, core_ids=list(range(NCORES)),
                               trace=True, trace_cores=trace_cores)
    if res.instructions_and_trace is not None:
        print("trace path:", res.instructions_and_trace[1])
    return res.exec_time_ns


# revision 22
# speedup vs baseline: 1.0158x; 1.0121x over previous
"""Distributed volume-argmin (correlation volume max/argmax) on 8 NeuronCores.

Problem: feat_l, feat_r [1, 128, 96, 128] fp32.
  corr[b,h,w,ij] = sum_c feat_l[b,c,h,w] * feat_r[b,c,i,j]
  flow_cost = max over ij, flat = argmax over ij,
  flow = (xoff - (flat % W_R)*scale_x, yoff - (flat // W_R)*scale_y)

Sharding: the 96 left-image rows are split across 8 cores (12 rows = 1536
left pixels each); feat_r is replicated. Each core computes its
[1536 x 12288] correlation block (K=C=128 contraction on partitions) and a
full per-pixel max/argmax, so no cross-core reduction is needed — outputs
concatenate.

Matmul precision: inputs are split host-side into bf16 hi/lo pairs and each
512-chunk accumulates hi*hi + hi*lo + lo*hi in fp32 PSUM (fp32-grade
accuracy at 3x-bf16-pass cost).

Argmax strategy per 128-pixel tile (partition = left pixel, free = 12288
right pixels): a ternary-then-binary fold tree on the Vector engine.
The 12288 row is split in thirds T1|T2|U; U (groups 4,5) is evicted
PSUM->SBUF by ScalarE (x2), then
  cma[j] = max(T2[j] (PSUM), x2[j])         j in [0,4096)
  cm1[j] = max(T1[j] (PSUM), cma[j])
  cm2[j] = max(cm1[j], cm1[j+2048])
  cm3[j] = max(cm2[j], cm2[j+1024])
so the DVE folds double as the PSUM drain. MAX8 + FIND_INDEX8 on the
1024-wide cm3 give the exact max M and the fold residue i*. The fold
branch bits are recovered exactly on the Scalar engine with its free
sum-accumulator: count = sum_j sign(M - y[j]) over y in {x2, cma,
cm1_hi, cm2_hi} equals the array size minus 1 exactly when M lives in
that array (sign(0)=0 on HW, verified); flat = base + 2048*b + 1024*c
+ i* with base = 8192 if M in U else 4096 if M in T2|U else 0. All
values are exact fp32 throughout, and ties resolve like jnp.argmax
(first occurrence) because corr maxima are unique for continuous data.
"""

import sys

for p in ("/opt/trn_rl_repo",):
    if p not in sys.path:
        sys.path.insert(0, p)

import numpy as np
import ml_dtypes

import concourse.bass as bass
import concourse.tile as tile
from concourse import bacc, mybir
from concourse.bass_utils import run_bass_kernel_spmd

# Problem geometry (hardcoded per the task contract).
B, C, H, W = 1, 128, 96, 128
H_R, W_R = 96, 128
NPIX = H_R * W_R              # 12288 right pixels
NCORES = 8
ROWS_PER_CORE = H // NCORES   # 12 left rows per core
NT = ROWS_PER_CORE            # one tile per left row (128 pixels)
CHUNK = 512
GRP = 1024                    # psum group (2 banks)
NGRP = NPIX // GRP            # 6
WT = NPIX // 3               # third width: 4096 (ternary first level)
W2F = WT // 2                 # 2048 (fold2 output)
W3F = WT // 4                 # 1024 (fold3 output)

_F32 = mybir.dt.float32
_BF16 = mybir.dt.bfloat16
_U32 = mybir.dt.uint32
_FP8 = mybir.dt.float8e4

_compiled = {}


def _emit_group_mms(nc, ps_ap, lh, ll, rh_t, rl_t, local_base):
    """3-pass bf16 matmuls for the chunks of one psum group.

    Pass-major order (hi*hi, hi*lo, lo*hi across chunks) so consecutive
    matmuls write different PSUM banks — same-bank accumulation
    back-to-back serializes the PE's fill/drain pipeline (379 vs 216 ns).
    rh_t/rl_t are per-third rhs tiles; local_base is the column offset of
    this group within its third."""
    for pass_lhs, pass_rhs, st, sp in (
        (lh, rh_t, True, False),
        (lh, rl_t, False, False),
        (ll, rh_t, False, True),
    ):
        for s in range(GRP // CHUNK):
            col0 = local_base + s * CHUNK
            out = ps_ap[:, s * CHUNK:(s + 1) * CHUNK]
            nc.tensor.matmul(out, pass_lhs, pass_rhs[:, col0:col0 + CHUNK],
                             start=st, stop=sp)


def _build_nc():
    nc = bacc.Bacc("TRN2", target_bir_lowering=False, debug=False,
                   num_devices=NCORES)

    lhs_hi_d = nc.declare_dram_parameter("lhs_hi", [C, NT * 128], _BF16,
                                         isOutput=False)
    lhs_lo_d = nc.declare_dram_parameter("lhs_lo", [C, NT * 128], _BF16,
                                         isOutput=False)
    rhs_hi_d = nc.declare_dram_parameter("rhs_hi", [C, NPIX], _BF16,
                                         isOutput=False)
    rhs_lo_d = nc.declare_dram_parameter("rhs_lo", [C, NPIX], _BF16,
                                         isOutput=False)
    cost_d = nc.declare_dram_parameter("cost", [128, NT], _F32, isOutput=True)
    idx_d = nc.declare_dram_parameter("idx", [128, NT], _U32, isOutput=True)
    cntu_d = nc.declare_dram_parameter("cntu", [128, NT], _F32, isOutput=True)
    cnta_d = nc.declare_dram_parameter("cnta", [128, NT], _F32, isOutput=True)
    cntb_d = nc.declare_dram_parameter("cntb", [128, NT], _F32, isOutput=True)
    cntc_d = nc.declare_dram_parameter("cntc", [128, NT], _F32, isOutput=True)

    with tile.TileContext(nc) as tc:
        with (
            tc.tile_pool(name="rhs", bufs=1) as rhs_pool,
            tc.tile_pool(name="lhs", bufs=1) as lhs_pool,
            tc.tile_pool(name="x2p", bufs=2) as x2_pool,
            tc.tile_pool(name="cmap", bufs=2) as cma_pool,
            tc.tile_pool(name="cm1p", bufs=2) as cm1_pool,
            tc.tile_pool(name="cm2p", bufs=1) as cm2_pool,
            tc.tile_pool(name="cm3p", bufs=2) as cm3_pool,
            tc.tile_pool(name="dumpp", bufs=1) as dump_pool,
            tc.tile_pool(name="m8p", bufs=3) as m8_pool,
            tc.tile_pool(name="ps", bufs=4, space="PSUM") as ps_pool,
            tc.tile_pool(name="outcols", bufs=1) as out_pool,
        ):
            lhs_hi = lhs_pool.tile([C, NT * 128], _BF16, tag="lh")
            lhs_lo = lhs_pool.tile([C, NT * 128], _BF16, tag="ll")
            # per-third rhs tiles: Tile tracks DMA dependencies per tile, so
            # the first matmuls wait only for their own third's load instead
            # of the whole 6.3 MB rhs. hi via sync (HWDGE), lo via gpsimd
            # (SWDGE). Load order: U third (needed first), then T2, then T1.
            rh0 = rhs_pool.tile([C, WT], _BF16, tag="rh0")
            rh1 = rhs_pool.tile([C, WT], _BF16, tag="rh1")
            rh2 = rhs_pool.tile([C, WT], _BF16, tag="rh2")
            rl0 = rhs_pool.tile([C, WT], _BF16, tag="rl0")
            rl1 = rhs_pool.tile([C, WT], _BF16, tag="rl1")
            rl2 = rhs_pool.tile([C, WT], _BF16, tag="rl2")
            rhs_hi_T = [rh0, rh1, rh2]
            rhs_lo_T = [rl0, rl1, rl2]
            nc.sync.dma_start(rhs_hi_T[2][:], rhs_hi_d[:, 2 * WT:3 * WT])
            nc.sync.dma_start(lhs_hi[:], lhs_hi_d[:])
            nc.gpsimd.dma_start(rhs_lo_T[2][:], rhs_lo_d[:, 2 * WT:3 * WT])
            nc.gpsimd.dma_start(lhs_lo[:], lhs_lo_d[:])
            for i in (1, 0):
                nc.sync.dma_start(rhs_hi_T[i][:], rhs_hi_d[:, i * WT:(i + 1) * WT])
                nc.gpsimd.dma_start(rhs_lo_T[i][:], rhs_lo_d[:, i * WT:(i + 1) * WT])

            cost_cols = out_pool.tile([128, NT], _F32, tag="cc")
            idx_cols = out_pool.tile([128, NT], _U32, tag="ic")
            cntu_cols = out_pool.tile([128, NT], _F32, tag="cu")
            cnta_cols = out_pool.tile([128, NT], _F32, tag="ca")
            cntb_cols = out_pool.tile([128, NT], _F32, tag="cb")
            cntc_cols = out_pool.tile([128, NT], _F32, tag="ccx")

            dump = dump_pool.tile([128, WT], _FP8, tag="dump")

            prev = None  # (x2, cma, cm1, cm2, m8, t) of previous tile

            def emit_counts(state):
                # small counts first: they unblock cm1/cm2 slab reuse
                x2_p, cma_p, cm1_p, cm2_p, m8_p, tp = state
                nc.scalar.activation(
                    dump[:, 0:W2F], cm1_p[:, W2F:WT],
                    mybir.ActivationFunctionType.Sign,
                    bias=m8_p[:, 0:1], scale=-1.0,
                    accum_out=cntb_cols[:, tp:tp + 1])
                nc.scalar.activation(
                    dump[:, 0:W3F], cm2_p[:, W3F:W2F],
                    mybir.ActivationFunctionType.Sign,
                    bias=m8_p[:, 0:1], scale=-1.0,
                    accum_out=cntc_cols[:, tp:tp + 1])
                nc.scalar.activation(
                    dump[:, 0:WT], x2_p[:],
                    mybir.ActivationFunctionType.Sign,
                    bias=m8_p[:, 0:1], scale=-1.0,
                    accum_out=cntu_cols[:, tp:tp + 1])
                nc.scalar.activation(
                    dump[:, 0:WT], cma_p[:],
                    mybir.ActivationFunctionType.Sign,
                    bias=m8_p[:, 0:1], scale=-1.0,
                    accum_out=cnta_cols[:, tp:tp + 1])

            for t in range(NT):
                lh = lhs_hi[:, t * 128:(t + 1) * 128]
                ll = lhs_lo[:, t * 128:(t + 1) * 128]

                x2 = x2_pool.tile([128, WT], _F32, tag="x2")
                cma = cma_pool.tile([128, WT], _F32, tag="cma")
                cm1 = cm1_pool.tile([128, WT], _F32, tag="cm1")

                GPT = WT // GRP  # groups per third
                # last third (U): evict to x2
                for k in range(GPT):
                    ps = ps_pool.tile([128, GRP], _F32, tag="ps")
                    sl = slice(k * GRP, (k + 1) * GRP)
                    _emit_group_mms(nc, ps[:], lh, ll, rhs_hi_T[2][:],
                                    rhs_lo_T[2][:], k * GRP)
                    nc.scalar.copy(x2[:, sl], ps[:])

                if prev is not None:
                    emit_counts(prev)

                # middle third: fold against x2 -> cma
                for k in range(GPT):
                    ps = ps_pool.tile([128, GRP], _F32, tag="ps")
                    sl = slice(k * GRP, (k + 1) * GRP)
                    _emit_group_mms(nc, ps[:], lh, ll, rhs_hi_T[1][:],
                                    rhs_lo_T[1][:], k * GRP)
                    nc.vector.tensor_max(cma[:, sl], ps[:], x2[:, sl])

                # first third: fold against cma -> cm1
                for k in range(GPT):
                    ps = ps_pool.tile([128, GRP], _F32, tag="ps")
                    sl = slice(k * GRP, (k + 1) * GRP)
                    _emit_group_mms(nc, ps[:], lh, ll, rhs_hi_T[0][:],
                                    rhs_lo_T[0][:], k * GRP)
                    nc.vector.tensor_max(cm1[:, sl], ps[:], cma[:, sl])

                cm2 = cm2_pool.tile([128, W2F], _F32, tag="cm2")
                nc.vector.tensor_max(cm2[:], cm1[:, 0:W2F], cm1[:, W2F:WT])
                cm3 = cm3_pool.tile([128, W3F], _F32, tag="cm3")
                nc.vector.tensor_max(cm3[:], cm2[:, 0:W3F], cm2[:, W3F:W2F])

                m8 = m8_pool.tile([128, 8], _F32, tag="m8")
                i8 = m8_pool.tile([128, 8], _U32, tag="i8")
                nc.vector.max(m8[:], cm3[:])
                nc.vector.max_index(i8[:], m8[:], cm3[:])
                nc.vector.tensor_copy(cost_cols[:, t:t + 1], m8[:, 0:1])
                nc.vector.tensor_copy(idx_cols[:, t:t + 1], i8[:, 0:1])

                prev = (x2, cma, cm1, cm2, m8, t)

            emit_counts(prev)

            nc.sync.dma_start(cost_d[:], cost_cols[:])
            nc.sync.dma_start(idx_d[:], idx_cols[:])
            nc.sync.dma_start(cntu_d[:], cntu_cols[:])
            nc.sync.dma_start(cnta_d[:], cnta_cols[:])
            nc.sync.dma_start(cntb_d[:], cntb_cols[:])
            nc.sync.dma_start(cntc_d[:], cntc_cols[:])

    nc.finalize()
    return nc


def _get_nc():
    if "nc" not in _compiled:
        _compiled["nc"] = _build_nc()
    return _compiled["nc"]


def _split_bf16(x):
    hi = x.astype(ml_dtypes.bfloat16)
    lo = (x - hi.astype(np.float32)).astype(ml_dtypes.bfloat16)
    return hi, lo


def _make_in_maps(feat_l, feat_r):
    rhs = np.ascontiguousarray(feat_r.reshape(C, NPIX))
    rhs_hi, rhs_lo = _split_bf16(rhs)
    in_maps = []
    for k in range(NCORES):
        lhs = np.ascontiguousarray(
            feat_l[0, :, k * ROWS_PER_CORE:(k + 1) * ROWS_PER_CORE, :]
            .reshape(C, ROWS_PER_CORE * W))
        lhs_hi, lhs_lo = _split_bf16(lhs)
        in_maps.append({
            "lhs_hi": lhs_hi, "lhs_lo": lhs_lo,
            "rhs_hi": rhs_hi, "rhs_lo": rhs_lo,
        })
    return in_maps


def _decode(results, scale_x, scale_y):
    flow_cost = np.empty((B, H, W), np.float32)
    flat = np.empty((H, W), np.int64)
    for k, r in enumerate(results):
        rows = slice(k * ROWS_PER_CORE, (k + 1) * ROWS_PER_CORE)
        flow_cost[0, rows, :] = r["cost"].T
        aU = (WT - r["cntu"].T.astype(np.int64))
        aA = (WT - r["cnta"].T.astype(np.int64))
        b = (W2F - r["cntb"].T.astype(np.int64))
        c = (W3F - r["cntc"].T.astype(np.int64))
        i_res = r["idx"].T.astype(np.int64)
        base = np.where(aU > 0, 2 * WT, np.where(aA > 0, WT, 0))
        flat[rows, :] = i_res + W3F * c + W2F * b + base
    np.clip(flat, 0, NPIX - 1, out=flat)
    xoff = np.arange(W)
    yoff = np.arange(H)
    u = -((flat % W_R) * scale_x - xoff[None, :]).astype(np.float32)
    v = -((flat // W_R) * scale_y - yoff[:, None]).astype(np.float32)
    flow = np.stack([u, v], axis=2)[None]  # [1, H, W, 2]
    return flow, flow_cost


def kernel(feat_l, feat_r, scale_x, scale_y):
    feat_l = np.asarray(feat_l, dtype=np.float32)
    feat_r = np.asarray(feat_r, dtype=np.float32)
    nc = _get_nc()
    in_maps = _make_in_maps(feat_l, feat_r)
    res = run_bass_kernel_spmd(nc, in_maps, core_ids=list(range(NCORES)))
    return _decode(res.results, int(scale_x), int(scale_y))


def run_timed(np_inputs, trace_cores=None):
    """Run once with NTFF tracing enabled; returns exec_time_ns (or None)."""
    feat_l = np.asarray(np_inputs["feat_l"], dtype=np.float32)
    feat_r = np.asarray(np_inputs["feat_r"], dtype=np.float32)
    nc = _get_nc()
    in_maps = _make_in_maps(feat_l, feat_r)
    res = run_bass_kernel_spmd(nc, in_maps, core_ids=list(range(NCORES)),
                               trace=True, trace_cores=trace_cores)
    if res.instructions_and_trace is not None:
        print("trace path:", res.instructions_and_trace[1])
    return res.exec_time_ns


# revision 23
# speedup vs baseline: 1.0231x; 1.0072x over previous
"""Distributed volume-argmin (correlation volume max/argmax) on 8 NeuronCores.

Problem: feat_l, feat_r [1, 128, 96, 128] fp32.
  corr[b,h,w,ij] = sum_c feat_l[b,c,h,w] * feat_r[b,c,i,j]
  flow_cost = max over ij, flat = argmax over ij,
  flow = (xoff - (flat % W_R)*scale_x, yoff - (flat // W_R)*scale_y)

Sharding: the 96 left-image rows are split across 8 cores (12 rows = 1536
left pixels each); feat_r is replicated. Each core computes its
[1536 x 12288] correlation block (K=C=128 contraction on partitions) and a
full per-pixel max/argmax, so no cross-core reduction is needed — outputs
concatenate.

Matmul precision: inputs are split host-side into bf16 hi/lo pairs and each
512-chunk accumulates hi*hi + hi*lo + lo*hi in fp32 PSUM (fp32-grade
accuracy at 3x-bf16-pass cost).

Argmax strategy per 128-pixel tile (partition = left pixel, free = 12288
right pixels): a ternary-then-binary fold tree on the Vector engine.
The 12288 row is split in thirds T1|T2|U; U (groups 4,5) is evicted
PSUM->SBUF by ScalarE (x2), then
  cma[j] = max(T2[j] (PSUM), x2[j])         j in [0,4096)
  cm1[j] = max(T1[j] (PSUM), cma[j])
  cm2[j] = max(cm1[j], cm1[j+2048])
  cm3[j] = max(cm2[j], cm2[j+1024])
so the DVE folds double as the PSUM drain. MAX8 + FIND_INDEX8 on the
1024-wide cm3 give the exact max M and the fold residue i*. The fold
branch bits are recovered exactly on the Scalar engine with its free
sum-accumulator: count = sum_j sign(M - y[j]) over y in {x2, cma,
cm1_hi, cm2_hi} equals the array size minus 1 exactly when M lives in
that array (sign(0)=0 on HW, verified); flat = base + 2048*b + 1024*c
+ i* with base = 8192 if M in U else 4096 if M in T2|U else 0. All
values are exact fp32 throughout, and ties resolve like jnp.argmax
(first occurrence) because corr maxima are unique for continuous data.
"""

import sys

for p in ("/opt/trn_rl_repo",):
    if p not in sys.path:
        sys.path.insert(0, p)

import numpy as np
import ml_dtypes

import concourse.bass as bass
import concourse.tile as tile
from concourse import bacc, mybir
from concourse.bass_utils import run_bass_kernel_spmd

# Problem geometry (hardcoded per the task contract).
B, C, H, W = 1, 128, 96, 128
H_R, W_R = 96, 128
NPIX = H_R * W_R              # 12288 right pixels
NCORES = 8
ROWS_PER_CORE = H // NCORES   # 12 left rows per core
NT = ROWS_PER_CORE            # one tile per left row (128 pixels)
CHUNK = 512
GRP = 1024                    # psum group (2 banks)
NGRP = NPIX // GRP            # 6
WT = NPIX // 3               # third width: 4096 (ternary first level)
W2F = WT // 2                 # 2048 (fold2 output)
W3F = WT // 4                 # 1024 (fold3 output)

_F32 = mybir.dt.float32
_BF16 = mybir.dt.bfloat16
_U32 = mybir.dt.uint32
_FP8 = mybir.dt.float8e4

_compiled = {}


def _emit_group_mms(nc, ps_ap, lh, ll, rh_t, rl_t, local_base):
    """3-pass bf16 matmuls for the chunks of one psum group.

    Pass-major order (hi*hi, hi*lo, lo*hi across chunks) so consecutive
    matmuls write different PSUM banks — same-bank accumulation
    back-to-back serializes the PE's fill/drain pipeline (379 vs 216 ns).
    rh_t/rl_t are per-third rhs tiles; local_base is the column offset of
    this group within its third."""
    for pass_lhs, pass_rhs, st, sp in (
        (lh, rh_t, True, False),
        (lh, rl_t, False, False),
        (ll, rh_t, False, True),
    ):
        for s in range(GRP // CHUNK):
            col0 = local_base + s * CHUNK
            out = ps_ap[:, s * CHUNK:(s + 1) * CHUNK]
            nc.tensor.matmul(out, pass_lhs, pass_rhs[:, col0:col0 + CHUNK],
                             start=st, stop=sp)


def _build_nc():
    nc = bacc.Bacc("TRN2", target_bir_lowering=False, debug=False,
                   num_devices=NCORES)

    lhs_hi_d = nc.declare_dram_parameter("lhs_hi", [C, NT * 128], _BF16,
                                         isOutput=False)
    lhs_lo_d = nc.declare_dram_parameter("lhs_lo", [C, NT * 128], _BF16,
                                         isOutput=False)
    rhs_hi_d = nc.declare_dram_parameter("rhs_hi", [C, NPIX], _BF16,
                                         isOutput=False)
    rhs_lo_d = nc.declare_dram_parameter("rhs_lo", [C, NPIX], _BF16,
                                         isOutput=False)
    cost_d = nc.declare_dram_parameter("cost", [128, NT], _F32, isOutput=True)
    idx_d = nc.declare_dram_parameter("idx", [128, NT], _U32, isOutput=True)
    cntu_d = nc.declare_dram_parameter("cntu", [128, NT], _F32, isOutput=True)
    cnta_d = nc.declare_dram_parameter("cnta", [128, NT], _F32, isOutput=True)
    cntb_d = nc.declare_dram_parameter("cntb", [128, NT], _F32, isOutput=True)
    cntc_d = nc.declare_dram_parameter("cntc", [128, NT], _F32, isOutput=True)

    with tile.TileContext(nc) as tc:
        with (
            tc.tile_pool(name="rhs", bufs=1) as rhs_pool,
            tc.tile_pool(name="lhs", bufs=1) as lhs_pool,
            tc.tile_pool(name="x2p", bufs=2) as x2_pool,
            tc.tile_pool(name="cmap", bufs=2) as cma_pool,
            tc.tile_pool(name="cm1p", bufs=2) as cm1_pool,
            tc.tile_pool(name="cm2p", bufs=1) as cm2_pool,
            tc.tile_pool(name="cm3p", bufs=2) as cm3_pool,
            tc.tile_pool(name="dumpp", bufs=1) as dump_pool,
            tc.tile_pool(name="m8p", bufs=3) as m8_pool,
            tc.tile_pool(name="ps", bufs=4, space="PSUM") as ps_pool,
            tc.tile_pool(name="outcols", bufs=1) as out_pool,
        ):
            lhs_hi = lhs_pool.tile([C, NT * 128], _BF16, tag="lh")
            lhs_lo = lhs_pool.tile([C, NT * 128], _BF16, tag="ll")
            # per-third rhs tiles: Tile tracks DMA dependencies per tile, so
            # the first matmuls wait only for their own third's load instead
            # of the whole 6.3 MB rhs. hi via sync (HWDGE), lo via gpsimd
            # (SWDGE). Load order: U third (needed first), then T2, then T1.
            rh0 = rhs_pool.tile([C, WT], _BF16, tag="rh0")
            rh1 = rhs_pool.tile([C, WT], _BF16, tag="rh1")
            rh2 = rhs_pool.tile([C, WT], _BF16, tag="rh2")
            rl0 = rhs_pool.tile([C, WT], _BF16, tag="rl0")
            rl1 = rhs_pool.tile([C, WT], _BF16, tag="rl1")
            rl2 = rhs_pool.tile([C, WT], _BF16, tag="rl2")
            rhs_hi_T = [rh0, rh1, rh2]
            rhs_lo_T = [rl0, rl1, rl2]
            nc.sync.dma_start(rhs_hi_T[2][:], rhs_hi_d[:, 2 * WT:3 * WT])
            nc.sync.dma_start(lhs_hi[:], lhs_hi_d[:])
            nc.gpsimd.dma_start(rhs_lo_T[2][:], rhs_lo_d[:, 2 * WT:3 * WT])
            nc.gpsimd.dma_start(lhs_lo[:], lhs_lo_d[:])
            for i in (1, 0):
                nc.sync.dma_start(rhs_hi_T[i][:], rhs_hi_d[:, i * WT:(i + 1) * WT])
                nc.gpsimd.dma_start(rhs_lo_T[i][:], rhs_lo_d[:, i * WT:(i + 1) * WT])

            cost_cols = out_pool.tile([128, NT], _F32, tag="cc")
            idx_cols = out_pool.tile([128, NT], _U32, tag="ic")
            cntu_cols = out_pool.tile([128, NT], _F32, tag="cu")
            cnta_cols = out_pool.tile([128, NT], _F32, tag="ca")
            cntb_cols = out_pool.tile([128, NT], _F32, tag="cb")
            cntc_cols = out_pool.tile([128, NT], _F32, tag="ccx")

            dump = dump_pool.tile([128, WT], _FP8, tag="dump")

            # warm the PE clock gate (HAM) during the DMA prologue: the PE
            # otherwise sits idle ~13 us and then runs the first ~3.4 us of
            # real matmuls at the cold 1.2 GHz rate. These dummies have no
            # input dependencies (garbage fp8 reads, result discarded) and
            # issue from kernel start, so HAM is at 8/8 when real work lands.
            warm_ps = ps_pool.tile([128, GRP], _F32, tag="ps")
            for _w in range(48):
                nc.tensor.matmul(warm_ps[:, 0:CHUNK], dump[:, 0:128],
                                 dump[:, 0:CHUNK], start=True, stop=True)

            prev = None  # (x2, cma, cm1, cm2, m8, t) of previous tile

            def emit_counts(state):
                # small counts first: they unblock cm1/cm2 slab reuse
                x2_p, cma_p, cm1_p, cm2_p, m8_p, tp = state
                nc.scalar.activation(
                    dump[:, 0:W2F], cm1_p[:, W2F:WT],
                    mybir.ActivationFunctionType.Sign,
                    bias=m8_p[:, 0:1], scale=-1.0,
                    accum_out=cntb_cols[:, tp:tp + 1])
                nc.scalar.activation(
                    dump[:, 0:W3F], cm2_p[:, W3F:W2F],
                    mybir.ActivationFunctionType.Sign,
                    bias=m8_p[:, 0:1], scale=-1.0,
                    accum_out=cntc_cols[:, tp:tp + 1])
                nc.scalar.activation(
                    dump[:, 0:WT], x2_p[:],
                    mybir.ActivationFunctionType.Sign,
                    bias=m8_p[:, 0:1], scale=-1.0,
                    accum_out=cntu_cols[:, tp:tp + 1])
                nc.scalar.activation(
                    dump[:, 0:WT], cma_p[:],
                    mybir.ActivationFunctionType.Sign,
                    bias=m8_p[:, 0:1], scale=-1.0,
                    accum_out=cnta_cols[:, tp:tp + 1])

            for t in range(NT):
                lh = lhs_hi[:, t * 128:(t + 1) * 128]
                ll = lhs_lo[:, t * 128:(t + 1) * 128]

                x2 = x2_pool.tile([128, WT], _F32, tag="x2")
                cma = cma_pool.tile([128, WT], _F32, tag="cma")
                cm1 = cm1_pool.tile([128, WT], _F32, tag="cm1")

                GPT = WT // GRP  # groups per third
                # last third (U): evict to x2
                for k in range(GPT):
                    ps = ps_pool.tile([128, GRP], _F32, tag="ps")
                    sl = slice(k * GRP, (k + 1) * GRP)
                    _emit_group_mms(nc, ps[:], lh, ll, rhs_hi_T[2][:],
                                    rhs_lo_T[2][:], k * GRP)
                    nc.scalar.copy(x2[:, sl], ps[:])

                if prev is not None:
                    emit_counts(prev)

                # middle third: fold against x2 -> cma
                for k in range(GPT):
                    ps = ps_pool.tile([128, GRP], _F32, tag="ps")
                    sl = slice(k * GRP, (k + 1) * GRP)
                    _emit_group_mms(nc, ps[:], lh, ll, rhs_hi_T[1][:],
                                    rhs_lo_T[1][:], k * GRP)
                    nc.vector.tensor_max(cma[:, sl], ps[:], x2[:, sl])

                # first third: fold against cma -> cm1
                for k in range(GPT):
                    ps = ps_pool.tile([128, GRP], _F32, tag="ps")
                    sl = slice(k * GRP, (k + 1) * GRP)
                    _emit_group_mms(nc, ps[:], lh, ll, rhs_hi_T[0][:],
                                    rhs_lo_T[0][:], k * GRP)
                    nc.vector.tensor_max(cm1[:, sl], ps[:], cma[:, sl])

                cm2 = cm2_pool.tile([128, W2F], _F32, tag="cm2")
                nc.vector.tensor_max(cm2[:], cm1[:, 0:W2F], cm1[:, W2F:WT])
                cm3 = cm3_pool.tile([128, W3F], _F32, tag="cm3")
                nc.vector.tensor_max(cm3[:], cm2[:, 0:W3F], cm2[:, W3F:W2F])

                m8 = m8_pool.tile([128, 8], _F32, tag="m8")
                i8 = m8_pool.tile([128, 8], _U32, tag="i8")
                nc.vector.max(m8[:], cm3[:])
                nc.vector.max_index(i8[:], m8[:], cm3[:])
                nc.vector.tensor_copy(cost_cols[:, t:t + 1], m8[:, 0:1])
                nc.vector.tensor_copy(idx_cols[:, t:t + 1], i8[:, 0:1])

                prev = (x2, cma, cm1, cm2, m8, t)

            emit_counts(prev)

            nc.sync.dma_start(cost_d[:], cost_cols[:])
            nc.sync.dma_start(idx_d[:], idx_cols[:])
            nc.sync.dma_start(cntu_d[:], cntu_cols[:])
            nc.sync.dma_start(cnta_d[:], cnta_cols[:])
            nc.sync.dma_start(cntb_d[:], cntb_cols[:])
            nc.sync.dma_start(cntc_d[:], cntc_cols[:])

    nc.finalize()
    return nc


def _get_nc():
    if "nc" not in _compiled:
        _compiled["nc"] = _build_nc()
    return _compiled["nc"]


def _split_bf16(x):
    hi = x.astype(ml_dtypes.bfloat16)
    lo = (x - hi.astype(np.float32)).astype(ml_dtypes.bfloat16)
    return hi, lo


def _make_in_maps(feat_l, feat_r):
    rhs = np.ascontiguousarray(feat_r.reshape(C, NPIX))
    rhs_hi, rhs_lo = _split_bf16(rhs)
    in_maps = []
    for k in range(NCORES):
        lhs = np.ascontiguousarray(
            feat_l[0, :, k * ROWS_PER_CORE:(k + 1) * ROWS_PER_CORE, :]
            .reshape(C, ROWS_PER_CORE * W))
        lhs_hi, lhs_lo = _split_bf16(lhs)
        in_maps.append({
            "lhs_hi": lhs_hi, "lhs_lo": lhs_lo,
            "rhs_hi": rhs_hi, "rhs_lo": rhs_lo,
        })
    return in_maps


def _decode(results, scale_x, scale_y):
    flow_cost = np.empty((B, H, W), np.float32)
    flat = np.empty((H, W), np.int64)
    for k, r in enumerate(results):
        rows = slice(k * ROWS_PER_CORE, (k + 1) * ROWS_PER_CORE)
        flow_cost[0, rows, :] = r["cost"].T
        aU = (WT - r["cntu"].T.astype(np.int64))
        aA = (WT - r["cnta"].T.astype(np.int64))
        b = (W2F - r["cntb"].T.astype(np.int64))
        c = (W3F - r["cntc"].T.astype(np.int64))
        i_res = r["idx"].T.astype(np.int64)
        base = np.where(aU > 0, 2 * WT, np.where(aA > 0, WT, 0))
        flat[rows, :] = i_res + W3F * c + W2F * b + base
    np.clip(flat, 0, NPIX - 1, out=flat)
    xoff = np.arange(W)
    yoff = np.arange(H)
    u = -((flat % W_R) * scale_x - xoff[None, :]).astype(np.float32)
    v = -((flat // W_R) * scale_y - yoff[:, None]).astype(np.float32)
    flow = np.stack([u, v], axis=2)[None]  # [1, H, W, 2]
    return flow, flow_cost


def kernel(feat_l, feat_r, scale_x, scale_y):
    feat_l = np.asarray(feat_l, dtype=np.float32)
    feat_r = np.asarray(feat_r, dtype=np.float32)
    nc = _get_nc()
    in_maps = _make_in_maps(feat_l, feat_r)
    res = run_bass_kernel_spmd(nc, in_maps, core_ids=list(range(NCORES)))
    return _decode(res.results, int(scale_x), int(scale_y))


def run_timed(np_inputs, trace_cores=None):
    """Run once with NTFF tracing enabled; returns exec_time_ns (or None)."""
    feat_l = np.asarray(np_inputs["feat_l"], dtype=np.float32)
    feat_r = np.asarray(np_inputs["feat_r"], dtype=np.float32)
    nc = _get_nc()
    in_maps = _make_in_maps(feat_l, feat_r)
    res = run_bass_kernel_spmd(nc, in_maps, core_ids=list(range(NCORES)),
                               trace=True, trace_cores=trace_cores)
    if res.instructions_and_trace is not None:
        print("trace path:", res.instructions_and_trace[1])
    return res.exec_time_ns
